# revision 1
# baseline (speedup 1.0000x reference)
"""Trainium2 Bass kernel for an attentive LSTM cell.

Data-parallel across 8 NeuronCores: batch (64) is sharded 8 per core, all
weights replicated.  Per core, for each batch item the kernel streams the
[2048, 512] annotation matrix through SBUF in [512, 512] tiles:

  1. PE-transposes each tile block-wise (ann^T needed because the matmul
     contraction runs over the partition dim), staging in PSUM.
  2. uh^T = kernel_u^T @ ann^T accumulated in PSUM (float32r matmuls: full
     1 cycle/row rate at N=512, vs 4 cycles/row for plain fp32).
  3. tanh(uh + Wx + bias_u) fused on the scalar engine (per-partition bias).
  4. et = v . tanh(...) via a v-stationary matmul; exp on the scalar engine
     with the softmax denominator accumulated in the same instruction.
  5. context += w^T @ ann (natural-layout tile), normalized at the end.

The LSTM tail (z = x@W + h@R + b, gates, c/h update) runs batched over the
core's 8 rows with x^T/h^T assembled from tiny PE transposes.
"""

import os
import sys

for _p in ("/opt/trn_rl_repo", "/root/.axon_site/_ro/trn_rl_repo"):
    if os.path.isdir(_p) and _p not in sys.path:
        sys.path.insert(0, _p)

import numpy as np

import concourse.bass as bass
import concourse.mybir as mybir
import concourse.tile as tile
from concourse import bacc
from concourse.bass_utils import run_bass_kernel_spmd
from concourse.masks import make_identity

AF = mybir.ActivationFunctionType
F32 = mybir.dt.float32
F32R = mybir.dt.float32r
BF16 = mybir.dt.bfloat16
USE_BF16_ANN = True

N_CORES = 8
B, T, A, U, D = 64, 2048, 512, 512, 512
BS = B // N_CORES  # batch rows per core
TT = 512           # t macro-tile
NT = T // TT       # macro tiles per batch row
NS = TT // 128     # 128-row subtiles per macro tile
J = A // 128       # contraction chunks (annotation dim)
M = U // 128       # unit chunks


def _r(ap):
    return ap.bitcast(F32R)


def build_bass(stage="full", repeat=1):
    nc = bacc.Bacc(trn_type="TRN2", debug=False)

    ann_d = nc.dram_tensor("ann", [BS, T, A], F32, kind="ExternalInput").ap()
    inp_d = nc.dram_tensor("inputs", [BS, D], F32, kind="ExternalInput").ap()
    h_d = nc.dram_tensor("h", [BS, U], F32, kind="ExternalInput").ap()
    c_d = nc.dram_tensor("c", [BS, U], F32, kind="ExternalInput").ap()
    W_d = nc.dram_tensor("kernel", [D + A, 4 * U], F32, kind="ExternalInput").ap()
    R_d = nc.dram_tensor("rkernel", [U, 4 * U], F32, kind="ExternalInput").ap()
    bias_d = nc.dram_tensor("bias", [1, 6 * U], F32, kind="ExternalInput").ap()
    ku_d = nc.dram_tensor("ku", [A, U], F32, kind="ExternalInput").ap()
    kw_d = nc.dram_tensor("kw", [U, U], F32, kind="ExternalInput").ap()
    kv_d = nc.dram_tensor("kv", [1, U], F32, kind="ExternalInput").ap()
    out_d = nc.dram_tensor("out", [BS, U], F32, kind="ExternalOutput").ap()
    global _W_SCRATCH
    _W_SCRATCH = [nc.dram_tensor(f"wscratch{k}", [1, TT], F32R).ap()
                  for k in range(2)]

    with tile.TileContext(nc) as tc:
        if repeat > 1:
            with tc.For_i(0, repeat, 1):
                _body(nc, tc, ann_d, inp_d, h_d, c_d, W_d, R_d, bias_d, ku_d,
                      kw_d, kv_d, out_d, stage)
        else:
            _body(nc, tc, ann_d, inp_d, h_d, c_d, W_d, R_d, bias_d, ku_d,
                  kw_d, kv_d, out_d, stage)
    nc.compile()
    return nc


def _body(nc, tc, ann_d, inp_d, h_d, c_d, W_d, R_d, bias_d, ku_d, kw_d, kv_d,
          out_d, stage="full"):
    with (
        tc.tile_pool(name="const", bufs=1) as cpool,
        tc.tile_pool(name="wts", bufs=1) as wpool,
    ):
        ident = cpool.tile([128, 128], F32)
        make_identity(nc, ident)
        AT = BF16 if USE_BF16_ANN else F32R   # attention data dtype
        ident_t = cpool.tile([128, 128], BF16, name="ident_t") if USE_BF16_ANN else ident
        if USE_BF16_ANN:
            nc.vector.tensor_copy(ident_t, ident)
        ones11_t = cpool.tile([1, 1], BF16, name="ones11_t") if USE_BF16_ANN else None
        ident_r = cpool.tile([128, 128], F32R)
        nc.vector.tensor_copy(ident_r, ident)
        ones11 = cpool.tile([1, 1], F32)
        nc.vector.memset(ones11, 1.0)
        ones11_r = cpool.tile([1, 1], F32R)
        nc.vector.tensor_copy(ones11_r, ones11)
        if ones11_t is None:
            ones11_t = ones11_r
        else:
            nc.vector.tensor_copy(ones11_t, ones11)
        ones1b_ld = cpool.tile([1, BS], F32)
        nc.vector.memset(ones1b_ld, 1.0)
        ones1b = cpool.tile([1, BS], F32R)
        nc.vector.tensor_copy(ones1b, ones1b_ld)
        half_col = cpool.tile([BS, 1], F32)
        nc.vector.memset(half_col, 0.5)

        # --- replicated weights ---
        ku_ld = wpool.tile([128, J, U], F32)   # ku[a, u] -> [p, j, u], a=128j+p
        nc.sync.dma_start(out=ku_ld, in_=ku_d.rearrange("(j p) u -> p j u", p=128))
        ku_sb = wpool.tile([128, J, U], AT)
        nc.vector.tensor_copy(ku_sb, ku_ld)
        kw_ld = wpool.tile([128, J, U], F32)
        nc.sync.dma_start(out=kw_ld, in_=kw_d.rearrange("(j p) u -> p j u", p=128))
        kw_sb = wpool.tile([128, J, U], F32R)
        nc.vector.tensor_copy(kw_sb, kw_ld)
        v_ld = cpool.tile([128, M], F32)       # v[u] -> [p, m], u=128m+p
        nc.sync.dma_start(out=v_ld, in_=kv_d.rearrange("o (m p) -> p (o m)", p=128))
        v_col = cpool.tile([128, M], F32R)
        nc.vector.tensor_copy(v_col, v_ld)
        biasu_col = cpool.tile([128, M], F32)  # bias[4U:5U] as a column
        nc.sync.dma_start(
            out=biasu_col,
            in_=bias_d[:, 4 * U:5 * U].rearrange("o (m p) -> p (o m)", p=128))
        biasz_ld = cpool.tile([1, 4 * U], F32)
        nc.sync.dma_start(out=biasz_ld, in_=bias_d[:, 0:4 * U])
        biasz_row = cpool.tile([1, 4 * U], F32R)
        nc.vector.tensor_copy(biasz_row, biasz_ld)

        # --- per-core state rows ---
        h_nat = cpool.tile([BS, U], F32)
        nc.sync.dma_start(out=h_nat, in_=h_d)
        in_nat = cpool.tile([BS, D], F32)
        nc.sync.dma_start(out=in_nat, in_=inp_d)
        c_nat = cpool.tile([BS, U], F32)
        nc.sync.dma_start(out=c_nat, in_=c_d)

        hT = wpool.tile([128, M, BS], F32R)     # h^T, contraction layout
        xT = wpool.tile([128, 2 * J, BS], F32R)  # [inputs; context]^T
        bias_att = wpool.tile([128, M, BS], F32)  # Wx^T + bias_u per batch row

        with tc.tile_pool(name="ps_setup", bufs=2, space="PSUM") as pps:
            for j in range(M):
                pt = pps.tile([128, BS], F32)
                nc.tensor.transpose(pt, h_nat[:, 128 * j:128 * (j + 1)],
                                    ident[0:BS, 0:BS])
                nc.vector.tensor_copy(hT[:, j, :], pt)
            for j in range(J):
                pt = pps.tile([128, BS], F32)
                nc.tensor.transpose(pt, in_nat[:, 128 * j:128 * (j + 1)],
                                    ident[0:BS, 0:BS])
                nc.vector.tensor_copy(xT[:, j, :], pt)
            for m in range(M):
                pwx = pps.tile([128, BS], F32)
                for j in range(M):
                    nc.tensor.matmul(pwx,
                                     lhsT=kw_sb[:, j, 128 * m:128 * (m + 1)],
                                     rhs=hT[:, j, :],
                                     start=(j == 0), stop=(j == M - 1))
                nc.scalar.activation(bias_att[:, m, :], pwx, AF.Identity,
                                     bias=biasu_col[:, m:m + 1])

        dump = cpool.tile([BS, U], F32)
        nc.vector.memset(dump, 0.0)
        if stage == "setup":
            nc.vector.tensor_copy(dump[:, 0:BS], hT[0:BS, 0, :])
            nc.vector.tensor_copy(dump[:, BS:2 * BS], bias_att[0:BS, 0, :])
            nc.sync.dma_start(out=out_d, in_=dump)
            return

        # ------------- attention over the annotation stream -------------
        with (
            tc.tile_pool(name="ann", bufs=2) as annpool,
            tc.tile_pool(name="annT", bufs=2) as annTpool,
            tc.tile_pool(name="tanh", bufs=2) as tanhpool,
            tc.tile_pool(name="big_ps", bufs=3, space="PSUM") as bigps,
            tc.tile_pool(name="small_ps", bufs=2, space="PSUM") as smallps,
            tc.tile_pool(name="small_sb", bufs=2) as smallsb,
        ):
            for b in range(BS):
                ctx_acc = smallsb.tile([1, A], F32, tag="ctxacc")
                nc.vector.memset(ctx_acc, 0.0)
                denb = smallsb.tile([1, NT], F32, tag="den")
                for i in range(NT):
                    ann_t = annpool.tile([128, NS, A], F32)
                    nc.sync.dma_start(
                        out=ann_t,
                        in_=ann_d[b, TT * i:TT * (i + 1), :].rearrange(
                            "(s p) a -> p s a", p=128))
                    ann_r = annpool.tile([128, NS, A], AT, tag="ann_r")
                    nc.vector.tensor_copy(ann_r, ann_t)

                    annT = annTpool.tile([128, J, TT], AT)
                    tr_src = ann_r if USE_BF16_ANN else ann_t
                    tr_id = ident_t if USE_BF16_ANN else ident
                    stg_dt = BF16 if USE_BF16_ANN else F32
                    for j in range(J):
                        stg = bigps.tile([128, TT], stg_dt, tag="big")
                        for s in range(NS):
                            nc.tensor.transpose(
                                stg[:, 128 * s:128 * (s + 1)],
                                tr_src[:, s, 128 * j:128 * (j + 1)], tr_id)
                        if j % 2 == 0:
                            nc.scalar.activation(annT[:, j, :], stg, AF.Copy)
                        else:
                            nc.vector.tensor_copy(annT[:, j, :], stg)
                    if stage == "transp":
                        nc.vector.tensor_copy(dump, annT[0:BS, 0, :])
                        continue

                    tanhG = tanhpool.tile([128, M, TT], F32R)
                    for mg in range(M // 2):
                        gps = bigps.tile([128, 2, TT], F32, tag="big")
                        for mi in range(2):
                            m = 2 * mg + mi
                            for j in range(J):
                                nc.tensor.matmul(
                                    gps[:, mi, :],
                                    lhsT=ku_sb[:, j, 128 * m:128 * (m + 1)],
                                    rhs=annT[:, j, :],
                                    start=(j == 0), stop=(j == J - 1))
                            nc.scalar.activation(tanhG[:, m, :], gps[:, mi, :],
                                                 AF.Tanh,
                                                 bias=bias_att[:, m, b:b + 1])

                    if stage == "g":
                        nc.vector.tensor_copy(dump, tanhG[0:BS, 0, :])
                        continue

                    et_ps = smallps.tile([1, TT], F32, tag="sm")
                    for m in range(M):
                        nc.tensor.matmul(et_ps, lhsT=v_col[:, m:m + 1],
                                         rhs=tanhG[:, m, :],
                                         start=(m == 0), stop=(m == M - 1))
                    w_row = smallsb.tile([1, TT], AT, tag="wrow")
                    nc.scalar.activation(w_row, et_ps, AF.Exp,
                                         accum_out=denb[:, i:i + 1])

                    wcw = 2 if USE_BF16_ANN else 1  # pad bf16 cols to 4B
                    wc_ps = smallps.tile([128, NS * wcw], AT, tag="sm")
                    for s in range(NS):
                        nc.tensor.transpose(wc_ps[:, wcw * s:wcw * s + 1],
                                            w_row[:, 128 * s:128 * (s + 1)],
                                            ones11_t if USE_BF16_ANN else ones11_r)
                    w_col = smallsb.tile([128, NS], AT, tag="wcol")
                    if USE_BF16_ANN:
                        nc.vector.tensor_copy(
                            w_col, wc_ps.rearrange("p (s w) -> p s w", w=2)[:, :, 0])
                    else:
                        nc.vector.tensor_copy(w_col, wc_ps)

                    if stage == "et":
                        nc.vector.tensor_copy(dump[0:1, :], w_row)
                        continue

                    ctx_ps = smallps.tile([1, A], F32, tag="sm")
                    for s in range(NS):
                        nc.tensor.matmul(ctx_ps, lhsT=w_col[:, s:s + 1],
                                         rhs=ann_r[:, s, :],
                                         start=(s == 0), stop=(s == NS - 1))
                    nc.vector.tensor_add(ctx_acc, ctx_acc, ctx_ps)

                if stage in ("transp", "g", "et"):
                    continue
                # normalize context, transpose into xT[:, J:2J, b]
                dsum = smallsb.tile([1, 1], F32, tag="dsum")
                nc.vector.reduce_sum(dsum, denb, axis=mybir.AxisListType.X)
                drec = smallsb.tile([1, 1], F32, tag="drec")
                nc.vector.reciprocal(drec, dsum)
                ctx_row = smallsb.tile([1, A], F32, tag="ctxrow")
                nc.vector.tensor_scalar_mul(ctx_row, ctx_acc, drec)
                cT_ps = smallps.tile([128, J], F32, tag="sm")
                for j in range(J):
                    nc.tensor.transpose(cT_ps[:, j:j + 1],
                                        ctx_row[:, 128 * j:128 * (j + 1)],
                                        ones11)
                nc.vector.tensor_copy(xT[:, J:2 * J, b], cT_ps)
                if stage == "ctx":
                    nc.vector.tensor_copy(dump[0:1, :], ctx_row)

        if stage in ("transp", "g", "et", "ctx"):
            nc.sync.dma_start(out=out_d, in_=dump)
            return

        # ------------- LSTM tail, batched over the core's rows -------------
        with (
            tc.tile_pool(name="wstream", bufs=2) as wsp,
            tc.tile_pool(name="z_ps", bufs=2, space="PSUM") as zpool,
            tc.tile_pool(name="gates", bufs=1) as gpool,
        ):
            gates = []
            for n in range(4):
                Wn_ld = wsp.tile([128, 2 * J, U], F32, tag="wn_ld")
                nc.sync.dma_start(
                    out=Wn_ld,
                    in_=W_d[:, U * n:U * (n + 1)].rearrange(
                        "(k p) n -> p k n", p=128))
                Wn = wsp.tile([128, 2 * J, U], F32R, tag="wn")
                nc.vector.tensor_copy(Wn, Wn_ld)
                Rn_ld = wsp.tile([128, M, U], F32, tag="rn_ld")
                nc.sync.dma_start(
                    out=Rn_ld,
                    in_=R_d[:, U * n:U * (n + 1)].rearrange(
                        "(k p) n -> p k n", p=128))
                Rn = wsp.tile([128, M, U], F32R, tag="rn")
                nc.vector.tensor_copy(Rn, Rn_ld)
                zps = zpool.tile([BS, U], F32)
                for k in range(2 * J):
                    nc.tensor.matmul(zps, lhsT=xT[:, k, :],
                                     rhs=Wn[:, k, :],
                                     start=(k == 0), stop=False)
                for k in range(M):
                    nc.tensor.matmul(zps, lhsT=hT[:, k, :],
                                     rhs=Rn[:, k, :],
                                     start=False, stop=False)
                nc.tensor.matmul(zps, lhsT=ones1b,
                                 rhs=biasz_row[:, U * n:U * (n + 1)],
                                 start=False, stop=True)
                g = gpool.tile([BS, U], F32, tag=f"gate{n}")
                if n == 2:  # candidate cell state
                    nc.scalar.activation(g, zps, AF.Tanh)
                else:       # hard sigmoid: clip(0.2 z + 0.5, 0, 1)
                    nc.scalar.activation(g, zps, AF.Relu, bias=half_col,
                                         scale=0.2)
                    nc.vector.tensor_scalar_min(g, g, 1.0)
                gates.append(g)

            gi, gf, gg, go = gates
            c_new = gpool.tile([BS, U], F32, tag="cnew")
            nc.vector.tensor_mul(c_new, gf, c_nat)
            ig = gpool.tile([BS, U], F32, tag="ig")
            nc.vector.tensor_mul(ig, gi, gg)
            nc.vector.tensor_add(c_new, c_new, ig)
            tc_t = gpool.tile([BS, U], F32, tag="tanhc")
            nc.scalar.activation(tc_t, c_new, AF.Tanh)
            h_new = gpool.tile([BS, U], F32, tag="hnew")
            nc.vector.tensor_mul(h_new, go, tc_t)
            nc.sync.dma_start(out=out_d, in_=h_new)


_NC_CACHE = None


def _get_nc():
    global _NC_CACHE
    if _NC_CACHE is None:
        _NC_CACHE = build_bass()
    return _NC_CACHE


def make_in_maps(inputs, h, c, annotations, kernel, recurrent_kernel, bias,
                 kernel_u, kernel_w, kernel_v):
    asc = np.ascontiguousarray
    maps = []
    for core in range(N_CORES):
        sl = slice(core * BS, (core + 1) * BS)
        maps.append({
            "ann": asc(annotations[sl]).astype(np.float32),
            "inputs": asc(inputs[sl]).astype(np.float32),
            "h": asc(h[sl]).astype(np.float32),
            "c": asc(c[sl]).astype(np.float32),
            "kernel": asc(kernel).astype(np.float32),
            "rkernel": asc(recurrent_kernel).astype(np.float32),
            "bias": asc(bias).reshape(1, 6 * U).astype(np.float32),
            "ku": asc(kernel_u).astype(np.float32),
            "kw": asc(kernel_w).astype(np.float32),
            "kv": asc(kernel_v).reshape(1, U).astype(np.float32),
        })
    return maps


def kernel(inputs, h, c, annotations, kernel, recurrent_kernel, bias,
           kernel_u, kernel_w, kernel_v, _trace=False):
    nc = _get_nc()
    in_maps = make_in_maps(inputs, h, c, annotations, kernel,
                           recurrent_kernel, bias, kernel_u, kernel_w,
                           kernel_v)
    res = run_bass_kernel_spmd(nc, in_maps, list(range(N_CORES)),
                               trace=_trace)
    out = np.concatenate([res.results[i]["out"] for i in range(N_CORES)],
                         axis=0)
    if _trace:
        kernel.last_exec_time_ns = res.exec_time_ns
        kernel.last_results = res
    return out



# revision 4
# speedup vs baseline: 1.4374x; 1.4374x over previous
"""Trainium2 Bass kernel for an attentive LSTM cell.

Data-parallel across 8 NeuronCores: batch (64) sharded 8 rows/core, weights
replicated.  Per core:

  - annotations are cast fp32->bf16 during the DMA (SWDGE) and kept resident
    per batch row ([128, 16, 512] bf16, double buffered).
  - per [512, 512] macro tile: PE block-transposes ann (bf16, PSUM staged),
    uh^T = ku^T @ ann^T accumulated in PSUM, tanh fused on ACT with the
    per-batch Wx+bias column, et = v . tanh(...) via v-stationary matmuls,
    exp on ACT with the softmax denominator accumulated in-instruction.
  - the et/exp/w-col stage of tile k-1 is issued after the transposes and
    uh matmuls of tile k (software pipelining) so PE never stalls on ACT.
  - context = sum_s w_col[s] . ann[s] over the resident row, normalized once.
  - LSTM tail weights (12 MB) are cast to bf16 during DMA and prefetched in
    chunks interleaved into the batch loop; the tail runs batched bf16
    matmuls over the core's 8 rows.
"""

import os
import sys

for _p in ("/opt/trn_rl_repo", "/root/.axon_site/_ro/trn_rl_repo"):
    if os.path.isdir(_p) and _p not in sys.path:
        sys.path.insert(0, _p)

import numpy as np

import concourse.bass as bass
import concourse.mybir as mybir
import concourse.tile as tile
from concourse import bacc
from concourse.bass_utils import run_bass_kernel_spmd
from concourse.masks import make_identity

AF = mybir.ActivationFunctionType
F32 = mybir.dt.float32
F32R = mybir.dt.float32r
BF16 = mybir.dt.bfloat16
FP8 = mybir.dt.float8e4

USE_FP8_UH = True      # ku/annT in fp8e4m3, DoubleRow matmuls for uh
KU_SCALE = 64.0        # ku pre-scale before fp8 cast (values ~N(0, 0.02))

N_CORES = 8
B, T, A, U, D = 64, 2048, 512, 512, 512
BS = B // N_CORES  # batch rows per core
TT = 512           # t macro-tile
NT = T // TT       # macro tiles per batch row
NS = TT // 128     # 128-row subtiles per macro tile
J = A // 128       # contraction chunks (annotation dim)
M = U // 128       # unit chunks
TS = T // 128      # 128-row subtiles per full batch row


def build_bass(stage="full", repeat=1):
    nc = bacc.Bacc(trn_type="TRN2", debug=False)

    ann_d = nc.dram_tensor("ann", [BS, T, A], F32, kind="ExternalInput").ap()
    inp_d = nc.dram_tensor("inputs", [BS, D], F32, kind="ExternalInput").ap()
    h_d = nc.dram_tensor("h", [BS, U], F32, kind="ExternalInput").ap()
    c_d = nc.dram_tensor("c", [BS, U], F32, kind="ExternalInput").ap()
    W_d = nc.dram_tensor("kernel", [D + A, 4 * U], F32, kind="ExternalInput").ap()
    R_d = nc.dram_tensor("rkernel", [U, 4 * U], F32, kind="ExternalInput").ap()
    bias_d = nc.dram_tensor("bias", [1, 6 * U], F32, kind="ExternalInput").ap()
    ku_d = nc.dram_tensor("ku", [A, U], F32, kind="ExternalInput").ap()
    kw_d = nc.dram_tensor("kw", [U, U], F32, kind="ExternalInput").ap()
    kv_d = nc.dram_tensor("kv", [1, U], F32, kind="ExternalInput").ap()
    out_d = nc.dram_tensor("out", [BS, U], F32, kind="ExternalOutput").ap()

    with tile.TileContext(nc) as tc:
        if repeat > 1:
            with tc.For_i(0, repeat, 1):
                _body(nc, tc, ann_d, inp_d, h_d, c_d, W_d, R_d, bias_d, ku_d,
                      kw_d, kv_d, out_d, stage)
        else:
            _body(nc, tc, ann_d, inp_d, h_d, c_d, W_d, R_d, bias_d, ku_d,
                  kw_d, kv_d, out_d, stage)
    nc.compile()
    return nc


def _body(nc, tc, ann_d, inp_d, h_d, c_d, W_d, R_d, bias_d, ku_d, kw_d, kv_d,
          out_d, stage="full"):
    AT = FP8 if USE_FP8_UH else BF16  # dtype of annT / ku for the uh matmul
    with (
        tc.tile_pool(name="const", bufs=1) as cpool,
        tc.tile_pool(name="wts", bufs=1) as wpool,
    ):
        ident = cpool.tile([128, 128], F32)
        make_identity(nc, ident)
        ident_b = cpool.tile([128, 128], BF16)
        nc.vector.tensor_copy(ident_b, ident)
        ones11 = cpool.tile([1, 1], F32)
        nc.vector.memset(ones11, 1.0)
        ones11_b = cpool.tile([1, 1], BF16)
        nc.vector.tensor_copy(ones11_b, ones11)
        ones1b_ld = cpool.tile([1, BS], F32)
        nc.vector.memset(ones1b_ld, 1.0)
        ones1b = cpool.tile([1, BS], BF16)
        nc.vector.tensor_copy(ones1b, ones1b_ld)
        half_col = cpool.tile([BS, 1], F32)
        nc.vector.memset(half_col, 0.5)

        # --- replicated weights ---
        ku_ld = wpool.tile([128, J, U], F32)   # ku[a, u] -> [p, j, u], a=128j+p
        nc.sync.dma_start(out=ku_ld, in_=ku_d.rearrange("(j p) u -> p j u", p=128))
        ku_sb = wpool.tile([128, J, U], AT)
        if USE_FP8_UH:
            ku_sc = wpool.tile([128, J, U], F32, name="ku_sc")
            nc.vector.tensor_scalar_mul(ku_sc, ku_ld, KU_SCALE)
            nc.vector.tensor_copy(ku_sb, ku_sc)
        else:
            nc.vector.tensor_copy(ku_sb, ku_ld)
        kw_ld = wpool.tile([128, J, U], F32)
        nc.sync.dma_start(out=kw_ld, in_=kw_d.rearrange("(j p) u -> p j u", p=128))
        kw_sb = wpool.tile([128, J, U], BF16)
        nc.vector.tensor_copy(kw_sb, kw_ld)
        v_ld = cpool.tile([128, M], F32)       # v[u] -> [p, m], u=128m+p
        nc.sync.dma_start(out=v_ld, in_=kv_d.rearrange("o (m p) -> p (o m)", p=128))
        v_col = cpool.tile([128, M], F32R)
        nc.vector.tensor_copy(v_col, v_ld)
        biasu_col = cpool.tile([128, M], F32)  # bias[4U:5U] as a column
        nc.sync.dma_start(
            out=biasu_col,
            in_=bias_d[:, 4 * U:5 * U].rearrange("o (m p) -> p (o m)", p=128))
        biasz_ld = cpool.tile([1, 4 * U], F32)
        nc.sync.dma_start(out=biasz_ld, in_=bias_d[:, 0:4 * U])
        biasz_row = cpool.tile([1, 4 * U], BF16)
        nc.vector.tensor_copy(biasz_row, biasz_ld)

        # --- per-core state rows ---
        h_nat = cpool.tile([BS, U], F32)
        nc.sync.dma_start(out=h_nat, in_=h_d)
        in_nat = cpool.tile([BS, D], F32)
        nc.sync.dma_start(out=in_nat, in_=inp_d)
        c_nat = cpool.tile([BS, U], F32)
        nc.sync.dma_start(out=c_nat, in_=c_d)

        hT = wpool.tile([128, M, BS], BF16)      # h^T, contraction layout
        xT = wpool.tile([128, 2 * J, BS], BF16)  # [inputs; context]^T
        bias_att = wpool.tile([128, M, BS], F32)  # Wx^T + bias_u per batch row

        with tc.tile_pool(name="ps_setup", bufs=2, space="PSUM") as pps:
            for j in range(M):
                pt = pps.tile([128, BS], F32)
                nc.tensor.transpose(pt, h_nat[:, 128 * j:128 * (j + 1)],
                                    ident[0:BS, 0:BS])
                nc.vector.tensor_copy(hT[:, j, :], pt)
            for j in range(J):
                pt = pps.tile([128, BS], F32)
                nc.tensor.transpose(pt, in_nat[:, 128 * j:128 * (j + 1)],
                                    ident[0:BS, 0:BS])
                nc.vector.tensor_copy(xT[:, j, :], pt)
            for m in range(M):
                pwx = pps.tile([128, BS], F32)
                for j in range(M):
                    nc.tensor.matmul(pwx,
                                     lhsT=kw_sb[:, j, 128 * m:128 * (m + 1)],
                                     rhs=hT[:, j, :],
                                     start=(j == 0), stop=(j == M - 1))
                nc.scalar.activation(bias_att[:, m, :], pwx, AF.Identity,
                                     bias=biasu_col[:, m:m + 1])

        # LSTM tail weights, bf16 via SWDGE cast-DMA, prefetched in chunks
        # interleaved into the batch loop below (one chunk per b iteration).
        Wt = wpool.tile([128, 2 * J, 4 * U], BF16)  # W[k, n] -> [p, k, n]
        Rt = wpool.tile([128, M, 4 * U], BF16)
        w_chunks = []
        for n in range(4):
            w_chunks.append((Wt[:, :, U * n:U * (n + 1)],
                             W_d[:, U * n:U * (n + 1)].rearrange(
                                 "(k p) n -> p k n", p=128)))
            w_chunks.append((Rt[:, :, U * n:U * (n + 1)],
                             R_d[:, U * n:U * (n + 1)].rearrange(
                                 "(k p) n -> p k n", p=128)))

        # ------------- attention over the annotation stream -------------
        with (
            tc.tile_pool(name="annres", bufs=2) as annpool,
            tc.tile_pool(name="annT", bufs=2) as annTpool,
            tc.tile_pool(name="tanh", bufs=2) as tanhpool,
            tc.tile_pool(name="stg_ps", bufs=2, space="PSUM") as stgps,
            tc.tile_pool(name="uh_ps", bufs=2, space="PSUM") as uhps,
            tc.tile_pool(name="small_ps", bufs=2, space="PSUM") as smallps,
            tc.tile_pool(name="small_sb", bufs=2) as smallsb,
        ):
            for b in range(BS):
                # whole-row annotation load, fp32->bf16 cast in the DMA
                ann_res = annpool.tile([128, TS, A], BF16, tag="annres")
                nc.gpsimd.dma_start(
                    out=ann_res,
                    in_=ann_d[b].rearrange("(q p) a -> p q a", p=128))
                # one tail-weight chunk per b iteration rides the same queue
                if b < len(w_chunks):
                    nc.gpsimd.dma_start(out=w_chunks[b][0], in_=w_chunks[b][1])

                denb = smallsb.tile([1, NT], F32, tag="den")
                w_cols = smallsb.tile([128, NT, NS], BF16, tag="wcols")
                pend = None  # (tanhG, et-psum-slot, i) of the previous tile

                def et_stage(pend_tanhG, i):
                    et_ps = smallps.tile([1, TT], F32, tag="sm")
                    for m in range(M):
                        nc.tensor.matmul(et_ps, lhsT=v_col[:, m:m + 1],
                                         rhs=pend_tanhG[:, m, :],
                                         start=(m == 0), stop=(m == M - 1))
                    w_row = smallsb.tile([1, TT], BF16, tag="wrow")
                    nc.scalar.activation(w_row, et_ps, AF.Exp,
                                         accum_out=denb[:, i:i + 1])
                    wc_ps = smallps.tile([128, NS * 2], BF16, tag="sm")
                    for s in range(NS):
                        nc.tensor.transpose(wc_ps[:, 2 * s:2 * s + 1],
                                            w_row[:, 128 * s:128 * (s + 1)],
                                            ones11_b)
                    nc.vector.tensor_copy(
                        w_cols[:, i, :],
                        wc_ps.rearrange("p (s w) -> p s w", w=2)[:, :, 0])

                for i in range(NT):
                    # PE: block-transposes of tile i (bf16 via PSUM)
                    annT = annTpool.tile([128, J, TT], AT)
                    for j in range(J):
                        stg = stgps.tile([128, TT], BF16, tag="stg")
                        for s in range(NS):
                            nc.tensor.transpose(
                                stg[:, 128 * s:128 * (s + 1)],
                                ann_res[:, NS * i + s, 128 * j:128 * (j + 1)],
                                ident_b)
                        nc.vector.tensor_copy(annT[:, j, :], stg)

                    # PE: uh^T accumulation + ACT tanh, per 128-unit chunk
                    tanhG = tanhpool.tile([128, M, TT], F32R)
                    for m in range(M):
                        uh = uhps.tile([128, TT], F32, tag="uh")
                        if USE_FP8_UH:
                            for g in range(J // 2):
                                nc.tensor.matmul(
                                    uh,
                                    lhsT=ku_sb[:, 2 * g:2 * g + 2,
                                               128 * m:128 * (m + 1)],
                                    rhs=annT[:, 2 * g:2 * g + 2, :],
                                    start=(g == 0), stop=(g == J // 2 - 1),
                                    perf_mode=mybir.MatmulPerfMode.DoubleRow)
                            nc.scalar.activation(
                                tanhG[:, m, :], uh, AF.Tanh,
                                bias=bias_att[:, m, b:b + 1],
                                scale=1.0 / KU_SCALE)
                        else:
                            for j in range(J):
                                nc.tensor.matmul(
                                    uh,
                                    lhsT=ku_sb[:, j, 128 * m:128 * (m + 1)],
                                    rhs=annT[:, j, :],
                                    start=(j == 0), stop=(j == J - 1))
                            nc.scalar.activation(
                                tanhG[:, m, :], uh, AF.Tanh,
                                bias=bias_att[:, m, b:b + 1])

                    # software-pipelined et/exp/w-col of the previous tile
                    if pend is not None:
                        et_stage(pend[0], pend[1])
                    pend = (tanhG, i)

                et_stage(pend[0], pend[1])

                # softmax denominator and context over the resident row
                dsum = smallsb.tile([1, 1], F32, tag="dsum")
                nc.vector.reduce_sum(dsum, denb, axis=mybir.AxisListType.X)
                drec = smallsb.tile([1, 1], F32, tag="drec")
                nc.vector.reciprocal(drec, dsum)
                ctx_ps = smallps.tile([1, A], F32, tag="sm")
                for q in range(TS):
                    nc.tensor.matmul(ctx_ps,
                                     lhsT=w_cols[:, q // NS, q % NS:q % NS + 1],
                                     rhs=ann_res[:, q, :],
                                     start=(q == 0), stop=(q == TS - 1))
                ctx_row = smallsb.tile([1, A], F32, tag="ctxrow")
                nc.vector.tensor_scalar_mul(ctx_row, ctx_ps, drec)
                cT_ps = smallps.tile([128, J], F32, tag="sm")
                for j in range(J):
                    nc.tensor.transpose(cT_ps[:, j:j + 1],
                                        ctx_row[:, 128 * j:128 * (j + 1)],
                                        ones11)
                nc.vector.tensor_copy(xT[:, J:2 * J, b], cT_ps)

        # ------------- LSTM tail, batched over the core's rows -------------
        with (
            tc.tile_pool(name="z_ps", bufs=2, space="PSUM") as zpool,
            tc.tile_pool(name="gates", bufs=1) as gpool,
        ):
            gates = []
            for n in range(4):
                zps = zpool.tile([BS, U], F32)
                for k in range(2 * J):
                    nc.tensor.matmul(zps, lhsT=xT[:, k, :],
                                     rhs=Wt[:, k, U * n:U * (n + 1)],
                                     start=(k == 0), stop=False)
                for k in range(M):
                    nc.tensor.matmul(zps, lhsT=hT[:, k, :],
                                     rhs=Rt[:, k, U * n:U * (n + 1)],
                                     start=False, stop=False)
                nc.tensor.matmul(zps, lhsT=ones1b,
                                 rhs=biasz_row[:, U * n:U * (n + 1)],
                                 start=False, stop=True)
                g = gpool.tile([BS, U], F32, tag=f"gate{n}")
                if n == 2:  # candidate cell state
                    nc.scalar.activation(g, zps, AF.Tanh)
                else:       # hard sigmoid: clip(0.2 z + 0.5, 0, 1)
                    nc.scalar.activation(g, zps, AF.Relu, bias=half_col,
                                         scale=0.2)
                    nc.vector.tensor_scalar_min(g, g, 1.0)
                gates.append(g)

            gi, gf, gg, go = gates
            c_new = gpool.tile([BS, U], F32, tag="cnew")
            nc.vector.tensor_mul(c_new, gf, c_nat)
            ig = gpool.tile([BS, U], F32, tag="ig")
            nc.vector.tensor_mul(ig, gi, gg)
            nc.vector.tensor_add(c_new, c_new, ig)
            tc_t = gpool.tile([BS, U], F32, tag="tanhc")
            nc.scalar.activation(tc_t, c_new, AF.Tanh)
            h_new = gpool.tile([BS, U], F32, tag="hnew")
            nc.vector.tensor_mul(h_new, go, tc_t)
            nc.sync.dma_start(out=out_d, in_=h_new)


_NC_CACHE = None


def _get_nc():
    global _NC_CACHE
    if _NC_CACHE is None:
        _NC_CACHE = build_bass()
    return _NC_CACHE


def make_in_maps(inputs, h, c, annotations, kernel, recurrent_kernel, bias,
                 kernel_u, kernel_w, kernel_v):
    asc = np.ascontiguousarray
    maps = []
    for core in range(N_CORES):
        sl = slice(core * BS, (core + 1) * BS)
        maps.append({
            "ann": asc(annotations[sl]).astype(np.float32),
            "inputs": asc(inputs[sl]).astype(np.float32),
            "h": asc(h[sl]).astype(np.float32),
            "c": asc(c[sl]).astype(np.float32),
            "kernel": asc(kernel).astype(np.float32),
            "rkernel": asc(recurrent_kernel).astype(np.float32),
            "bias": asc(bias).reshape(1, 6 * U).astype(np.float32),
            "ku": asc(kernel_u).astype(np.float32),
            "kw": asc(kernel_w).astype(np.float32),
            "kv": asc(kernel_v).reshape(1, U).astype(np.float32),
        })
    return maps


def kernel(inputs, h, c, annotations, kernel, recurrent_kernel, bias,
           kernel_u, kernel_w, kernel_v, _trace=False):
    nc = _get_nc()
    in_maps = make_in_maps(inputs, h, c, annotations, kernel,
                           recurrent_kernel, bias, kernel_u, kernel_w,
                           kernel_v)
    res = run_bass_kernel_spmd(nc, in_maps, list(range(N_CORES)),
                               trace=_trace)
    out = np.concatenate([res.results[i]["out"] for i in range(N_CORES)],
                         axis=0)
    if _trace:
        kernel.last_exec_time_ns = res.exec_time_ns
        kernel.last_results = res
    return out


# revision 6
# speedup vs baseline: 1.5050x; 1.0470x over previous
"""Trainium2 Bass kernel for an attentive LSTM cell — v4.

v3 + : m-outer uh/tanh (one bias column per unit-chunk -> N=1024 ACT
activations over the whole row), fp8 tanh output with DoubleRow et matmuls,
optional fp8 annotations with DoubleRow context matmuls, half-row annotation
DMAs, and a one-row software pipeline: per batch row b the PE does
transposes(b) -> uh(b) (tanh on ACT) -> et/ctx(b-1), so PE never waits on
the scalar engine.
"""

import os
import sys

for _p in ("/opt/trn_rl_repo", "/root/.axon_site/_ro/trn_rl_repo"):
    if os.path.isdir(_p) and _p not in sys.path:
        sys.path.insert(0, _p)

import numpy as np

import concourse.bass as bass
import concourse.mybir as mybir
import concourse.tile as tile
from concourse import bacc
from concourse.bass_utils import run_bass_kernel_spmd
from concourse.masks import make_identity

AF = mybir.ActivationFunctionType
DR = mybir.MatmulPerfMode.DoubleRow
F32 = mybir.dt.float32
F32R = mybir.dt.float32r
BF16 = mybir.dt.bfloat16
FP8 = mybir.dt.float8e4

ANN_FP8 = False        # fp8 resident annotations + DoubleRow context
KU_SCALE = 64.0        # ku pre-scale before fp8 cast (values ~N(0, 0.02))
V_SCALE = 64.0         # kv pre-scale before fp8 cast

N_CORES = 8
B, T, A, U, D = 64, 2048, 512, 512, 512
BS = B // N_CORES  # batch rows per core
TT = 512           # t macro-tile
NT = T // TT       # macro tiles per batch row
NS = TT // 128     # 128-row subtiles per macro tile
J = A // 128       # contraction chunks (annotation dim)
M = U // 128       # unit chunks
TS = T // 128      # 128-row subtiles per full batch row
HB = 1024          # tanh half-row width


def build_bass(stage="full", repeat=1):
    nc = bacc.Bacc(trn_type="TRN2", debug=False)

    ann_d = nc.dram_tensor("ann", [BS, T, A], F32, kind="ExternalInput").ap()
    inp_d = nc.dram_tensor("inputs", [BS, D], F32, kind="ExternalInput").ap()
    h_d = nc.dram_tensor("h", [BS, U], F32, kind="ExternalInput").ap()
    c_d = nc.dram_tensor("c", [BS, U], F32, kind="ExternalInput").ap()
    W_d = nc.dram_tensor("kernel", [D + A, 4 * U], F32, kind="ExternalInput").ap()
    R_d = nc.dram_tensor("rkernel", [U, 4 * U], F32, kind="ExternalInput").ap()
    bias_d = nc.dram_tensor("bias", [1, 6 * U], F32, kind="ExternalInput").ap()
    ku_d = nc.dram_tensor("ku", [A, U], F32, kind="ExternalInput").ap()
    kw_d = nc.dram_tensor("kw", [U, U], F32, kind="ExternalInput").ap()
    kv_d = nc.dram_tensor("kv", [1, U], F32, kind="ExternalInput").ap()
    out_d = nc.dram_tensor("out", [BS, U], F32, kind="ExternalOutput").ap()

    with tile.TileContext(nc) as tc:
        if repeat > 1:
            with tc.For_i(0, repeat, 1):
                _body(nc, tc, ann_d, inp_d, h_d, c_d, W_d, R_d, bias_d, ku_d,
                      kw_d, kv_d, out_d)
        else:
            _body(nc, tc, ann_d, inp_d, h_d, c_d, W_d, R_d, bias_d, ku_d,
                  kw_d, kv_d, out_d)
    nc.compile()
    return nc


def _body(nc, tc, ann_d, inp_d, h_d, c_d, W_d, R_d, bias_d, ku_d, kw_d, kv_d,
          out_d):
    ANT = FP8 if ANN_FP8 else BF16   # resident annotation dtype
    with (
        tc.tile_pool(name="const", bufs=1) as cpool,
        tc.tile_pool(name="wts", bufs=1) as wpool,
    ):
        ident = cpool.tile([128, 128], F32)
        make_identity(nc, ident)
        ident_t = cpool.tile([128, 128], ANT)
        nc.vector.tensor_copy(ident_t, ident)
        ones11 = cpool.tile([1, 1], F32)
        nc.vector.memset(ones11, 1.0)
        ones11_t = cpool.tile([1, 1], ANT)
        nc.vector.tensor_copy(ones11_t, ones11)
        ones1b_ld = cpool.tile([1, BS], F32)
        nc.vector.memset(ones1b_ld, 1.0)
        ones1b = cpool.tile([1, BS], BF16)
        nc.vector.tensor_copy(ones1b, ones1b_ld)
        half_col = cpool.tile([BS, 1], F32)
        nc.vector.memset(half_col, 0.5)

        # --- replicated weights ---
        ku_ld = wpool.tile([128, J, U], F32)   # ku[a, u] -> [p, j, u], a=128j+p
        nc.sync.dma_start(out=ku_ld, in_=ku_d.rearrange("(j p) u -> p j u", p=128))
        ku_sc = wpool.tile([128, J, U], F32, name="ku_sc")
        nc.vector.tensor_scalar_mul(ku_sc, ku_ld, KU_SCALE)
        ku_sb = wpool.tile([128, J, U], FP8)
        nc.vector.tensor_copy(ku_sb, ku_sc)
        kw_ld = wpool.tile([128, J, U], F32)
        nc.sync.dma_start(out=kw_ld, in_=kw_d.rearrange("(j p) u -> p j u", p=128))
        kw_sb = wpool.tile([128, J, U], BF16)
        nc.vector.tensor_copy(kw_sb, kw_ld)
        v_ld = cpool.tile([128, M], F32)       # v[u] -> [p, m], u=128m+p
        nc.sync.dma_start(out=v_ld, in_=kv_d.rearrange("o (m p) -> p (o m)", p=128))
        v_sc = cpool.tile([128, M], F32)
        nc.vector.tensor_scalar_mul(v_sc, v_ld, V_SCALE)
        v_pad = cpool.tile([128, M, 16], FP8)  # fp8 v, 16B-padded k-tile step
        nc.vector.memset(v_pad, 0.0)
        nc.vector.tensor_copy(v_pad[:, :, 0], v_sc)
        biasu_col = cpool.tile([128, M], F32)  # bias[4U:5U] as a column
        nc.sync.dma_start(
            out=biasu_col,
            in_=bias_d[:, 4 * U:5 * U].rearrange("o (m p) -> p (o m)", p=128))
        biasz_ld = cpool.tile([1, 4 * U], F32)
        nc.sync.dma_start(out=biasz_ld, in_=bias_d[:, 0:4 * U])
        biasz_row = cpool.tile([1, 4 * U], BF16)
        nc.vector.tensor_copy(biasz_row, biasz_ld)

        # --- per-core state rows ---
        h_nat = cpool.tile([BS, U], F32)
        nc.sync.dma_start(out=h_nat, in_=h_d)
        in_nat = cpool.tile([BS, D], F32)
        nc.sync.dma_start(out=in_nat, in_=inp_d)
        c_nat = cpool.tile([BS, U], F32)
        nc.sync.dma_start(out=c_nat, in_=c_d)

        hT = wpool.tile([128, M, BS], BF16)      # h^T, contraction layout
        xT = wpool.tile([128, 2 * J, BS], BF16)  # [inputs; context]^T
        bias_att = wpool.tile([128, M, BS], F32)  # Wx^T + bias_u per batch row

        with tc.tile_pool(name="ps_setup", bufs=2, space="PSUM") as pps:
            for j in range(M):
                pt = pps.tile([128, BS], F32)
                nc.tensor.transpose(pt, h_nat[:, 128 * j:128 * (j + 1)],
                                    ident[0:BS, 0:BS])
                nc.vector.tensor_copy(hT[:, j, :], pt)
            for j in range(J):
                pt = pps.tile([128, BS], F32)
                nc.tensor.transpose(pt, in_nat[:, 128 * j:128 * (j + 1)],
                                    ident[0:BS, 0:BS])
                nc.vector.tensor_copy(xT[:, j, :], pt)
            for m in range(M):
                pwx = pps.tile([128, BS], F32)
                for j in range(M):
                    nc.tensor.matmul(pwx,
                                     lhsT=kw_sb[:, j, 128 * m:128 * (m + 1)],
                                     rhs=hT[:, j, :],
                                     start=(j == 0), stop=(j == M - 1))
                nc.scalar.activation(bias_att[:, m, :], pwx, AF.Identity,
                                     bias=biasu_col[:, m:m + 1])

        # LSTM tail weights, bf16 via SWDGE cast-DMA, prefetched in chunks
        Wt = wpool.tile([128, 2 * J, 4 * U], BF16)
        Rt = wpool.tile([128, M, 4 * U], BF16)
        w_chunks = []
        for n in range(4):
            w_chunks.append((Wt[:, :, U * n:U * (n + 1)],
                             W_d[:, U * n:U * (n + 1)].rearrange(
                                 "(k p) n -> p k n", p=128)))
            w_chunks.append((Rt[:, :, U * n:U * (n + 1)],
                             R_d[:, U * n:U * (n + 1)].rearrange(
                                 "(k p) n -> p k n", p=128)))

        # ------------- attention -------------
        HQ = TS // 2  # row-half in 128-subtiles
        with (
            tc.tile_pool(name="annres", bufs=2) as annpool,
            tc.tile_pool(name="annT", bufs=2) as annTpool,
            tc.tile_pool(name="tanh", bufs=2) as tanhpool,
            tc.tile_pool(name="stg_ps", bufs=2, space="PSUM") as stgps,
            tc.tile_pool(name="uh_ps", bufs=2, space="PSUM") as uhps,
            tc.tile_pool(name="small_ps", bufs=2, space="PSUM") as smallps,
            tc.tile_pool(name="small_sb", bufs=2) as smallsb,
        ):
            pend = None

            def late_stage(p):
                # et / exp / w-cols / context for batch row b (one row late)
                b, tanhG, ann_halves = p
                denb = smallsb.tile([1, NT], F32, tag="den")
                w_cols = smallsb.tile([128, TS, 16], ANT, tag="wcols")
                for i in range(NT):
                    et_ps = smallps.tile([1, TT], F32, tag="sm")
                    for g in range(M // 2):
                        nc.tensor.matmul(
                            et_ps, lhsT=v_pad[:, 2 * g:2 * g + 2, 0:1],
                            rhs=tanhG[:, 2 * g:2 * g + 2,
                                      TT * i:TT * (i + 1)],
                            start=(g == 0), stop=(g == M // 2 - 1),
                            perf_mode=DR)
                    w_row = smallsb.tile([1, TT], ANT, tag="wrow")
                    nc.scalar.activation(w_row, et_ps, AF.Exp,
                                         scale=1.0 / V_SCALE,
                                         accum_out=denb[:, i:i + 1])
                    wc_ps = smallps.tile([128, NS * 4], ANT, tag="sm")
                    wcw = 4 if ANN_FP8 else 2  # pad cols to 4 bytes
                    for s in range(NS):
                        nc.tensor.transpose(wc_ps[:, wcw * s:wcw * s + 1],
                                            w_row[:, 128 * s:128 * (s + 1)],
                                            ones11_t)
                    nc.vector.tensor_copy(
                        w_cols[:, NS * i:NS * (i + 1), 0],
                        wc_ps.rearrange("p (s w) -> p s w", w=wcw)[:, 0:NS, 0])
                dsum = smallsb.tile([1, 1], F32, tag="dsum")
                nc.vector.reduce_sum(dsum, denb, axis=mybir.AxisListType.X)
                drec = smallsb.tile([1, 1], F32, tag="drec")
                nc.vector.reciprocal(drec, dsum)
                ctx_ps = smallps.tile([1, A], F32, tag="sm")
                if ANN_FP8:
                    for h in range(2):
                        for g in range(HQ // 2):
                            nc.tensor.matmul(
                                ctx_ps,
                                lhsT=w_cols[:, HQ * h + 2 * g:
                                            HQ * h + 2 * g + 2, 0:1],
                                rhs=ann_halves[h][:, 2 * g:2 * g + 2, :],
                                start=(h == 0 and g == 0),
                                stop=(h == 1 and g == HQ // 2 - 1),
                                perf_mode=DR)
                else:
                    for q in range(TS):
                        nc.tensor.matmul(
                            ctx_ps, lhsT=w_cols[:, q, 0:1],
                            rhs=ann_halves[q // HQ][:, q % HQ, :],
                            start=(q == 0), stop=(q == TS - 1))
                ctx_row = smallsb.tile([1, A], F32, tag="ctxrow")
                nc.vector.tensor_scalar_mul(ctx_row, ctx_ps, drec)
                cT_ps = smallps.tile([128, J], F32, tag="sm")
                for j in range(J):
                    nc.tensor.transpose(cT_ps[:, j:j + 1],
                                        ctx_row[:, 128 * j:128 * (j + 1)],
                                        ones11)
                nc.vector.tensor_copy(xT[:, J:2 * J, b], cT_ps)

            for b in range(BS):
                ann_halves = []
                for h in range(2):
                    ah = annpool.tile([128, HQ, A], ANT, tag=f"annres{h}")
                    nc.gpsimd.dma_start(
                        out=ah,
                        in_=ann_d[b, T // 2 * h:T // 2 * (h + 1), :]
                        .rearrange("(q p) a -> p q a", p=128))
                    ann_halves.append(ah)
                if b < len(w_chunks):
                    nc.gpsimd.dma_start(out=w_chunks[b][0], in_=w_chunks[b][1])

                # Per half-row: transpose burst then uh/tanh burst, so plain
                # matmuls pulse on PE at a sub-3.4us cadence (HAM warmth).
                # Transposes are REGULAR bf16 matmuls (out = ann_chunk.T @ I)
                # rather than transpose-mode, which the HAM activity monitor
                # does not count as PE-busy.
                annT = annTpool.tile([128, J, T], FP8)
                tanhG = tanhpool.tile([128, M, T], FP8)
                for h in range(2):
                    for i in range(HQ // NS):
                        for j in range(J):
                            stg = stgps.tile([128, TT], F32, tag="stg")
                            for s in range(NS):
                                nc.tensor.matmul(
                                    stg[:, 128 * s:128 * (s + 1)],
                                    lhsT=ann_halves[h][:, NS * i + s,
                                                       128 * j:128 * (j + 1)],
                                    rhs=ident_t,
                                    start=True, stop=True)
                            nc.vector.tensor_copy(
                                annT[:, j, T // 2 * h + TT * i:
                                     T // 2 * h + TT * (i + 1)], stg)
                    for m in range(M):
                        uh = uhps.tile([128, HB], F32, tag="uh")
                        for c in range(2):
                            for g in range(J // 2):
                                nc.tensor.matmul(
                                    uh[:, TT * c:TT * (c + 1)],
                                    lhsT=ku_sb[:, 2 * g:2 * g + 2,
                                               128 * m:128 * (m + 1)],
                                    rhs=annT[:, 2 * g:2 * g + 2,
                                             HB * h + TT * c:
                                             HB * h + TT * (c + 1)],
                                    start=(g == 0), stop=(g == J // 2 - 1),
                                    perf_mode=DR)
                        nc.scalar.activation(
                            tanhG[:, m, HB * h:HB * (h + 1)], uh, AF.Tanh,
                            bias=bias_att[:, m, b:b + 1],
                            scale=1.0 / KU_SCALE)
                    if h == 0 and pend is not None:
                        late_stage(pend)
                        pend = None

                pend = (b, tanhG, ann_halves)

            late_stage(pend)

        # ------------- LSTM tail -------------
        with (
            tc.tile_pool(name="z_ps", bufs=2, space="PSUM") as zpool,
            tc.tile_pool(name="gates", bufs=1) as gpool,
        ):
            gates = []
            for n in range(4):
                zps = zpool.tile([BS, U], F32)
                for k in range(2 * J):
                    nc.tensor.matmul(zps, lhsT=xT[:, k, :],
                                     rhs=Wt[:, k, U * n:U * (n + 1)],
                                     start=(k == 0), stop=False)
                for k in range(M):
                    nc.tensor.matmul(zps, lhsT=hT[:, k, :],
                                     rhs=Rt[:, k, U * n:U * (n + 1)],
                                     start=False, stop=False)
                nc.tensor.matmul(zps, lhsT=ones1b,
                                 rhs=biasz_row[:, U * n:U * (n + 1)],
                                 start=False, stop=True)
                g = gpool.tile([BS, U], F32, tag=f"gate{n}")
                if n == 2:
                    nc.scalar.activation(g, zps, AF.Tanh)
                else:
                    nc.scalar.activation(g, zps, AF.Relu, bias=half_col,
                                         scale=0.2)
                    nc.vector.tensor_scalar_min(g, g, 1.0)
                gates.append(g)

            gi, gf, gg, go = gates
            c_new = gpool.tile([BS, U], F32, tag="cnew")
            nc.vector.tensor_mul(c_new, gf, c_nat)
            ig = gpool.tile([BS, U], F32, tag="ig")
            nc.vector.tensor_mul(ig, gi, gg)
            nc.vector.tensor_add(c_new, c_new, ig)
            tc_t = gpool.tile([BS, U], F32, tag="tanhc")
            nc.scalar.activation(tc_t, c_new, AF.Tanh)
            h_new = gpool.tile([BS, U], F32, tag="hnew")
            nc.vector.tensor_mul(h_new, go, tc_t)
            nc.sync.dma_start(out=out_d, in_=h_new)


_NC_CACHE = None


def _get_nc():
    global _NC_CACHE
    if _NC_CACHE is None:
        _NC_CACHE = build_bass()
    return _NC_CACHE


def make_in_maps(inputs, h, c, annotations, kernel, recurrent_kernel, bias,
                 kernel_u, kernel_w, kernel_v):
    asc = np.ascontiguousarray
    maps = []
    for core in range(N_CORES):
        sl = slice(core * BS, (core + 1) * BS)
        maps.append({
            "ann": asc(annotations[sl]).astype(np.float32),
            "inputs": asc(inputs[sl]).astype(np.float32),
            "h": asc(h[sl]).astype(np.float32),
            "c": asc(c[sl]).astype(np.float32),
            "kernel": asc(kernel).astype(np.float32),
            "rkernel": asc(recurrent_kernel).astype(np.float32),
            "bias": asc(bias).reshape(1, 6 * U).astype(np.float32),
            "ku": asc(kernel_u).astype(np.float32),
            "kw": asc(kernel_w).astype(np.float32),
            "kv": asc(kernel_v).reshape(1, U).astype(np.float32),
        })
    return maps


def kernel(inputs, h, c, annotations, kernel, recurrent_kernel, bias,
           kernel_u, kernel_w, kernel_v, _trace=False):
    nc = _get_nc()
    in_maps = make_in_maps(inputs, h, c, annotations, kernel,
                           recurrent_kernel, bias, kernel_u, kernel_w,
                           kernel_v)
    res = run_bass_kernel_spmd(nc, in_maps, list(range(N_CORES)),
                               trace=_trace)
    out = np.concatenate([res.results[i]["out"] for i in range(N_CORES)],
                         axis=0)
    if _trace:
        globals()["last_exec_time_ns"] = res.exec_time_ns
        globals()["last_results"] = res
    return out


# revision 8
# speedup vs baseline: 2.0894x; 1.3884x over previous
"""Trainium2 Bass kernel for an attentive LSTM cell — v4.

v3 + : m-outer uh/tanh (one bias column per unit-chunk -> N=1024 ACT
activations over the whole row), fp8 tanh output with DoubleRow et matmuls,
optional fp8 annotations with DoubleRow context matmuls, half-row annotation
DMAs, and a one-row software pipeline: per batch row b the PE does
transposes(b) -> uh(b) (tanh on ACT) -> et/ctx(b-1), so PE never waits on
the scalar engine.
"""

import os
import sys

for _p in ("/opt/trn_rl_repo", "/root/.axon_site/_ro/trn_rl_repo"):
    if os.path.isdir(_p) and _p not in sys.path:
        sys.path.insert(0, _p)

import numpy as np

import concourse.bass as bass
import concourse.mybir as mybir
import concourse.tile as tile
from concourse import bacc
from concourse.bass_utils import run_bass_kernel_spmd
from concourse.masks import make_identity

AF = mybir.ActivationFunctionType
DR = mybir.MatmulPerfMode.DoubleRow
F32 = mybir.dt.float32
F32R = mybir.dt.float32r
BF16 = mybir.dt.bfloat16
FP8 = mybir.dt.float8e4

ANN_FP8 = False        # fp8 resident annotations + DoubleRow context
KU_SCALE = 64.0        # ku pre-scale before fp8 cast (values ~N(0, 0.02))
V_SCALE = 64.0         # kv pre-scale before fp8 cast

N_CORES = 8
B, T, A, U, D = 64, 2048, 512, 512, 512
BS = B // N_CORES  # batch rows per core
TT = 512           # t macro-tile
NT = T // TT       # macro tiles per batch row
NS = TT // 128     # 128-row subtiles per macro tile
J = A // 128       # contraction chunks (annotation dim)
M = U // 128       # unit chunks
TS = T // 128      # 128-row subtiles per full batch row
HB = 1024          # tanh half-row width


def build_bass(stage="full", repeat=1):
    nc = bacc.Bacc(trn_type="TRN2", debug=False)

    ann_d = nc.dram_tensor("ann", [BS, T, A], F32, kind="ExternalInput").ap()
    inp_d = nc.dram_tensor("inputs", [BS, D], F32, kind="ExternalInput").ap()
    h_d = nc.dram_tensor("h", [BS, U], F32, kind="ExternalInput").ap()
    c_d = nc.dram_tensor("c", [BS, U], F32, kind="ExternalInput").ap()
    W_d = nc.dram_tensor("kernel", [D + A, 4 * U], F32, kind="ExternalInput").ap()
    R_d = nc.dram_tensor("rkernel", [U, 4 * U], F32, kind="ExternalInput").ap()
    bias_d = nc.dram_tensor("bias", [1, 6 * U], F32, kind="ExternalInput").ap()
    ku_d = nc.dram_tensor("ku", [A, U], F32, kind="ExternalInput").ap()
    kw_d = nc.dram_tensor("kw", [U, U], F32, kind="ExternalInput").ap()
    kv_d = nc.dram_tensor("kv", [1, U], F32, kind="ExternalInput").ap()
    out_d = nc.dram_tensor("out", [BS, U], F32, kind="ExternalOutput").ap()

    with tile.TileContext(nc) as tc:
        if repeat > 1:
            with tc.For_i(0, repeat, 1):
                _body(nc, tc, ann_d, inp_d, h_d, c_d, W_d, R_d, bias_d, ku_d,
                      kw_d, kv_d, out_d)
        else:
            _body(nc, tc, ann_d, inp_d, h_d, c_d, W_d, R_d, bias_d, ku_d,
                  kw_d, kv_d, out_d)
    nc.compile()
    return nc


def _body(nc, tc, ann_d, inp_d, h_d, c_d, W_d, R_d, bias_d, ku_d, kw_d, kv_d,
          out_d):
    ANT = FP8 if ANN_FP8 else BF16   # resident annotation dtype
    with (
        tc.tile_pool(name="const", bufs=1) as cpool,
        tc.tile_pool(name="wts", bufs=1) as wpool,
    ):
        ident = cpool.tile([128, 128], F32)
        make_identity(nc, ident)
        ident_t = cpool.tile([128, 128], ANT)
        nc.vector.tensor_copy(ident_t, ident)
        ones11 = cpool.tile([1, 1], F32)
        nc.vector.memset(ones11, 1.0)
        ones11_t = cpool.tile([1, 1], ANT)
        nc.vector.tensor_copy(ones11_t, ones11)
        ones1b_ld = cpool.tile([1, BS], F32)
        nc.vector.memset(ones1b_ld, 1.0)
        ones1b = cpool.tile([1, BS], BF16)
        nc.vector.tensor_copy(ones1b, ones1b_ld)
        half_col = cpool.tile([BS, 1], F32)
        nc.vector.memset(half_col, 0.5)

        # --- replicated weights ---
        ku_ld = wpool.tile([128, J, U], F32)   # ku[a, u] -> [p, j, u], a=128j+p
        nc.sync.dma_start(out=ku_ld, in_=ku_d.rearrange("(j p) u -> p j u", p=128))
        ku_sc = wpool.tile([128, J, U], F32, name="ku_sc")
        nc.vector.tensor_scalar_mul(ku_sc, ku_ld, KU_SCALE)
        ku_sb = wpool.tile([128, J, U], FP8)
        nc.vector.tensor_copy(ku_sb, ku_sc)
        kw_ld = wpool.tile([128, J, U], F32)
        nc.sync.dma_start(out=kw_ld, in_=kw_d.rearrange("(j p) u -> p j u", p=128))
        kw_sb = wpool.tile([128, J, U], BF16)
        nc.vector.tensor_copy(kw_sb, kw_ld)
        v_ld = cpool.tile([128, M], F32)       # v[u] -> [p, m], u=128m+p
        nc.sync.dma_start(out=v_ld, in_=kv_d.rearrange("o (m p) -> p (o m)", p=128))
        v_sc = cpool.tile([128, M], F32)
        nc.vector.tensor_scalar_mul(v_sc, v_ld, V_SCALE)
        v_pad = cpool.tile([128, M, 16], FP8)  # fp8 v, 16B-padded k-tile step
        nc.vector.memset(v_pad, 0.0)
        nc.vector.tensor_copy(v_pad[:, :, 0], v_sc)
        biasu_col = cpool.tile([128, M], F32)  # bias[4U:5U] as a column
        nc.sync.dma_start(
            out=biasu_col,
            in_=bias_d[:, 4 * U:5 * U].rearrange("o (m p) -> p (o m)", p=128))
        biasz_ld = cpool.tile([1, 4 * U], F32)
        nc.sync.dma_start(out=biasz_ld, in_=bias_d[:, 0:4 * U])
        biasz_row = cpool.tile([1, 4 * U], BF16)
        nc.vector.tensor_copy(biasz_row, biasz_ld)

        # --- per-core state rows ---
        h_nat = cpool.tile([BS, U], F32)
        nc.sync.dma_start(out=h_nat, in_=h_d)
        in_nat = cpool.tile([BS, D], F32)
        nc.sync.dma_start(out=in_nat, in_=inp_d)
        c_nat = cpool.tile([BS, U], F32)
        nc.sync.dma_start(out=c_nat, in_=c_d)

        hT = wpool.tile([128, M, BS], BF16)      # h^T, contraction layout
        xT = wpool.tile([128, 2 * J, BS], BF16)  # [inputs; context]^T
        bias_att = wpool.tile([128, M, BS], F32)  # Wx^T + bias_u per batch row

        with tc.tile_pool(name="ps_setup", bufs=2, space="PSUM") as pps:
            for j in range(M):
                pt = pps.tile([128, BS], F32)
                nc.tensor.transpose(pt, h_nat[:, 128 * j:128 * (j + 1)],
                                    ident[0:BS, 0:BS])
                nc.vector.tensor_copy(hT[:, j, :], pt)
            for j in range(J):
                pt = pps.tile([128, BS], F32)
                nc.tensor.transpose(pt, in_nat[:, 128 * j:128 * (j + 1)],
                                    ident[0:BS, 0:BS])
                nc.vector.tensor_copy(xT[:, j, :], pt)
            for m in range(M):
                pwx = pps.tile([128, BS], F32)
                for j in range(M):
                    nc.tensor.matmul(pwx,
                                     lhsT=kw_sb[:, j, 128 * m:128 * (m + 1)],
                                     rhs=hT[:, j, :],
                                     start=(j == 0), stop=(j == M - 1))
                nc.scalar.activation(bias_att[:, m, :], pwx, AF.Identity,
                                     bias=biasu_col[:, m:m + 1])

        # LSTM tail weights, bf16 via SWDGE cast-DMA, prefetched in chunks
        Wt = wpool.tile([128, 2 * J, 4 * U], BF16)
        Rt = wpool.tile([128, M, 4 * U], BF16)
        w_chunks = []
        for n in range(4):
            w_chunks.append((Wt[:, :, U * n:U * (n + 1)],
                             W_d[:, U * n:U * (n + 1)].rearrange(
                                 "(k p) n -> p k n", p=128)))
            w_chunks.append((Rt[:, :, U * n:U * (n + 1)],
                             R_d[:, U * n:U * (n + 1)].rearrange(
                                 "(k p) n -> p k n", p=128)))

        # ------------- attention -------------
        HQ = TS // 2  # row-half in 128-subtiles
        with (
            tc.tile_pool(name="annres", bufs=3) as annpool,
            tc.tile_pool(name="annT", bufs=2) as annTpool,
            tc.tile_pool(name="tanh", bufs=2) as tanhpool,
            tc.tile_pool(name="stg_ps", bufs=2, space="PSUM") as stgps,
            tc.tile_pool(name="uh_ps", bufs=2, space="PSUM") as uhps,
            tc.tile_pool(name="small_ps", bufs=2, space="PSUM") as smallps,
            tc.tile_pool(name="small_sb", bufs=2) as smallsb,
        ):
            pend = None

            def late_stage(p):
                # et / exp / w-cols / context for batch row b (one row late)
                b, tanhG, ann_halves = p
                denb = smallsb.tile([1, NT], F32, tag="den")
                w_cols = smallsb.tile([128, TS, 16], ANT, tag="wcols")
                for i in range(NT):
                    et_ps = smallps.tile([1, TT], F32, tag="sm")
                    for g in range(M // 2):
                        nc.tensor.matmul(
                            et_ps, lhsT=v_pad[:, 2 * g:2 * g + 2, 0:1],
                            rhs=tanhG[:, 2 * g:2 * g + 2,
                                      TT * i:TT * (i + 1)],
                            start=(g == 0), stop=(g == M // 2 - 1),
                            perf_mode=DR)
                    w_row = smallsb.tile([1, TT], ANT, tag="wrow")
                    nc.scalar.activation(w_row, et_ps, AF.Exp,
                                         scale=1.0 / V_SCALE,
                                         accum_out=denb[:, i:i + 1])
                    wc_ps = smallps.tile([128, NS * 4], ANT, tag="sm")
                    wcw = 4 if ANN_FP8 else 2  # pad cols to 4 bytes
                    for s in range(NS):
                        nc.tensor.transpose(wc_ps[:, wcw * s:wcw * s + 1],
                                            w_row[:, 128 * s:128 * (s + 1)],
                                            ones11_t)
                    nc.vector.tensor_copy(
                        w_cols[:, NS * i:NS * (i + 1), 0],
                        wc_ps.rearrange("p (s w) -> p s w", w=wcw)[:, 0:NS, 0])
                dsum = smallsb.tile([1, 1], F32, tag="dsum")
                nc.vector.reduce_sum(dsum, denb, axis=mybir.AxisListType.X)
                drec = smallsb.tile([1, 1], F32, tag="drec")
                nc.vector.reciprocal(drec, dsum)
                ctx_ps = smallps.tile([1, A], F32, tag="sm")
                if ANN_FP8:
                    for h in range(2):
                        for g in range(HQ // 2):
                            nc.tensor.matmul(
                                ctx_ps,
                                lhsT=w_cols[:, HQ * h + 2 * g:
                                            HQ * h + 2 * g + 2, 0:1],
                                rhs=ann_halves[h][:, 2 * g:2 * g + 2, :],
                                start=(h == 0 and g == 0),
                                stop=(h == 1 and g == HQ // 2 - 1),
                                perf_mode=DR)
                else:
                    for q in range(TS):
                        nc.tensor.matmul(
                            ctx_ps, lhsT=w_cols[:, q, 0:1],
                            rhs=ann_halves[q // HQ][:, q % HQ, :],
                            start=(q == 0), stop=(q == TS - 1))
                ctx_row = smallsb.tile([1, A], F32, tag="ctxrow")
                nc.vector.tensor_scalar_mul(ctx_row, ctx_ps, drec)
                cT_ps = smallps.tile([128, J], F32, tag="sm")
                for j in range(J):
                    nc.tensor.transpose(cT_ps[:, j:j + 1],
                                        ctx_row[:, 128 * j:128 * (j + 1)],
                                        ones11)
                nc.vector.tensor_copy(xT[:, J:2 * J, b], cT_ps)

            def issue_ann_dma(b):
                halves = []
                for h in range(2):
                    ah = annpool.tile([128, HQ, A], ANT, tag=f"annres{h}")
                    nc.gpsimd.dma_start(
                        out=ah,
                        in_=ann_d[b, T // 2 * h:T // 2 * (h + 1), :]
                        .rearrange("(q p) a -> p q a", p=128))
                    halves.append(ah)
                return halves

            ann_next = issue_ann_dma(0)
            for b in range(BS):
                ann_halves = ann_next
                if b + 1 < BS:
                    ann_next = issue_ann_dma(b + 1)
                if b < len(w_chunks):
                    nc.gpsimd.dma_start(out=w_chunks[b][0], in_=w_chunks[b][1])

                # Per half-row: transpose burst then uh/tanh burst, so plain
                # matmuls pulse on PE at a sub-3.4us cadence (HAM warmth).
                # Transposes are REGULAR bf16 matmuls (out = ann_chunk.T @ I)
                # rather than transpose-mode, which the HAM activity monitor
                # does not count as PE-busy.
                annT = annTpool.tile([128, J, T], FP8)
                tanhG = tanhpool.tile([128, M, T], FP8)
                for h in range(2):
                    for i in range(HQ // NS):
                        for j in range(J):
                            stg = stgps.tile([128, TT], F32, tag="stg")
                            for s in range(NS):
                                nc.tensor.matmul(
                                    stg[:, 128 * s:128 * (s + 1)],
                                    lhsT=ann_halves[h][:, NS * i + s,
                                                       128 * j:128 * (j + 1)],
                                    rhs=ident_t,
                                    start=True, stop=True)
                            nc.vector.tensor_copy(
                                annT[:, j, T // 2 * h + TT * i:
                                     T // 2 * h + TT * (i + 1)], stg)
                    for m in range(M):
                        uh = uhps.tile([128, HB], F32, tag="uh")
                        for c in range(2):
                            for g in range(J // 2):
                                nc.tensor.matmul(
                                    uh[:, TT * c:TT * (c + 1)],
                                    lhsT=ku_sb[:, 2 * g:2 * g + 2,
                                               128 * m:128 * (m + 1)],
                                    rhs=annT[:, 2 * g:2 * g + 2,
                                             HB * h + TT * c:
                                             HB * h + TT * (c + 1)],
                                    start=(g == 0), stop=(g == J // 2 - 1),
                                    perf_mode=DR)
                        nc.scalar.activation(
                            tanhG[:, m, HB * h:HB * (h + 1)], uh, AF.Tanh,
                            bias=bias_att[:, m, b:b + 1],
                            scale=1.0 / KU_SCALE)
                    if h == 0 and pend is not None:
                        late_stage(pend)
                        pend = None

                pend = (b, tanhG, ann_halves)

            late_stage(pend)

        # ------------- LSTM tail -------------
        with (
            tc.tile_pool(name="z_ps", bufs=2, space="PSUM") as zpool,
            tc.tile_pool(name="gates", bufs=1) as gpool,
        ):
            gates = []
            for n in range(4):
                zps = zpool.tile([BS, U], F32)
                for k in range(2 * J):
                    nc.tensor.matmul(zps, lhsT=xT[:, k, :],
                                     rhs=Wt[:, k, U * n:U * (n + 1)],
                                     start=(k == 0), stop=False)
                for k in range(M):
                    nc.tensor.matmul(zps, lhsT=hT[:, k, :],
                                     rhs=Rt[:, k, U * n:U * (n + 1)],
                                     start=False, stop=False)
                nc.tensor.matmul(zps, lhsT=ones1b,
                                 rhs=biasz_row[:, U * n:U * (n + 1)],
                                 start=False, stop=True)
                g = gpool.tile([BS, U], F32, tag=f"gate{n}")
                if n == 2:
                    nc.scalar.activation(g, zps, AF.Tanh)
                else:
                    nc.scalar.activation(g, zps, AF.Relu, bias=half_col,
                                         scale=0.2)
                    nc.vector.tensor_scalar_min(g, g, 1.0)
                gates.append(g)

            gi, gf, gg, go = gates
            c_new = gpool.tile([BS, U], F32, tag="cnew")
            nc.vector.tensor_mul(c_new, gf, c_nat)
            ig = gpool.tile([BS, U], F32, tag="ig")
            nc.vector.tensor_mul(ig, gi, gg)
            nc.vector.tensor_add(c_new, c_new, ig)
            tc_t = gpool.tile([BS, U], F32, tag="tanhc")
            nc.scalar.activation(tc_t, c_new, AF.Tanh)
            h_new = gpool.tile([BS, U], F32, tag="hnew")
            nc.vector.tensor_mul(h_new, go, tc_t)
            nc.sync.dma_start(out=out_d, in_=h_new)


_NC_CACHE = None


def _get_nc():
    global _NC_CACHE
    if _NC_CACHE is None:
        _NC_CACHE = build_bass()
    return _NC_CACHE


def make_in_maps(inputs, h, c, annotations, kernel, recurrent_kernel, bias,
                 kernel_u, kernel_w, kernel_v):
    asc = np.ascontiguousarray
    maps = []
    for core in range(N_CORES):
        sl = slice(core * BS, (core + 1) * BS)
        maps.append({
            "ann": asc(annotations[sl]).astype(np.float32),
            "inputs": asc(inputs[sl]).astype(np.float32),
            "h": asc(h[sl]).astype(np.float32),
            "c": asc(c[sl]).astype(np.float32),
            "kernel": asc(kernel).astype(np.float32),
            "rkernel": asc(recurrent_kernel).astype(np.float32),
            "bias": asc(bias).reshape(1, 6 * U).astype(np.float32),
            "ku": asc(kernel_u).astype(np.float32),
            "kw": asc(kernel_w).astype(np.float32),
            "kv": asc(kernel_v).reshape(1, U).astype(np.float32),
        })
    return maps


def kernel(inputs, h, c, annotations, kernel, recurrent_kernel, bias,
           kernel_u, kernel_w, kernel_v, _trace=False):
    nc = _get_nc()
    in_maps = make_in_maps(inputs, h, c, annotations, kernel,
                           recurrent_kernel, bias, kernel_u, kernel_w,
                           kernel_v)
    res = run_bass_kernel_spmd(nc, in_maps, list(range(N_CORES)),
                               trace=_trace)
    out = np.concatenate([res.results[i]["out"] for i in range(N_CORES)],
                         axis=0)
    if _trace:
        globals()["last_exec_time_ns"] = res.exec_time_ns
        globals()["last_results"] = res
    return out


# revision 13
# speedup vs baseline: 2.1395x; 1.0240x over previous
"""Trainium2 Bass kernel for an attentive LSTM cell — v4.

v3 + : m-outer uh/tanh (one bias column per unit-chunk -> N=1024 ACT
activations over the whole row), fp8 tanh output with DoubleRow et matmuls,
optional fp8 annotations with DoubleRow context matmuls, half-row annotation
DMAs, and a one-row software pipeline: per batch row b the PE does
transposes(b) -> uh(b) (tanh on ACT) -> et/ctx(b-1), so PE never waits on
the scalar engine.
"""

import os
import sys

for _p in ("/opt/trn_rl_repo", "/root/.axon_site/_ro/trn_rl_repo"):
    if os.path.isdir(_p) and _p not in sys.path:
        sys.path.insert(0, _p)

import numpy as np

import concourse.bass as bass
import concourse.mybir as mybir
import concourse.tile as tile
from concourse import bacc
from concourse.bass_utils import run_bass_kernel_spmd
from concourse.masks import make_identity

AF = mybir.ActivationFunctionType
DR = mybir.MatmulPerfMode.DoubleRow
F32 = mybir.dt.float32
F32R = mybir.dt.float32r
BF16 = mybir.dt.bfloat16
FP8 = mybir.dt.float8e4

ANN_FP8 = False        # fp8 resident annotations + DoubleRow context
KU_SCALE = 64.0        # ku pre-scale before fp8 cast (values ~N(0, 0.02))
V_SCALE = 64.0         # kv pre-scale before fp8 cast

N_CORES = 8
B, T, A, U, D = 64, 2048, 512, 512, 512
BS = B // N_CORES  # batch rows per core
TT = 512           # t macro-tile
NT = T // TT       # macro tiles per batch row
NS = TT // 128     # 128-row subtiles per macro tile
J = A // 128       # contraction chunks (annotation dim)
M = U // 128       # unit chunks
TS = T // 128      # 128-row subtiles per full batch row
HB = 1024          # tanh half-row width


def build_bass(stage="full", repeat=1):
    nc = bacc.Bacc(trn_type="TRN2", debug=False)

    ann_d = nc.dram_tensor("ann", [BS, T, A], F32, kind="ExternalInput").ap()
    inp_d = nc.dram_tensor("inputs", [BS, D], F32, kind="ExternalInput").ap()
    h_d = nc.dram_tensor("h", [BS, U], F32, kind="ExternalInput").ap()
    c_d = nc.dram_tensor("c", [BS, U], F32, kind="ExternalInput").ap()
    W_d = nc.dram_tensor("kernel", [D + A, 4 * U], F32, kind="ExternalInput").ap()
    R_d = nc.dram_tensor("rkernel", [U, 4 * U], F32, kind="ExternalInput").ap()
    bias_d = nc.dram_tensor("bias", [1, 6 * U], F32, kind="ExternalInput").ap()
    ku_d = nc.dram_tensor("ku", [A, U], F32, kind="ExternalInput").ap()
    kw_d = nc.dram_tensor("kw", [U, U], F32, kind="ExternalInput").ap()
    kv_d = nc.dram_tensor("kv", [1, U], F32, kind="ExternalInput").ap()
    out_d = nc.dram_tensor("out", [BS, U], F32, kind="ExternalOutput").ap()

    with tile.TileContext(nc) as tc:
        if repeat > 1:
            with tc.For_i(0, repeat, 1):
                _body(nc, tc, ann_d, inp_d, h_d, c_d, W_d, R_d, bias_d, ku_d,
                      kw_d, kv_d, out_d)
        else:
            _body(nc, tc, ann_d, inp_d, h_d, c_d, W_d, R_d, bias_d, ku_d,
                  kw_d, kv_d, out_d)
    nc.compile()
    return nc


def _body(nc, tc, ann_d, inp_d, h_d, c_d, W_d, R_d, bias_d, ku_d, kw_d, kv_d,
          out_d):
    ANT = FP8 if ANN_FP8 else BF16   # resident annotation dtype
    HQ = TS // 2  # row-half in 128-subtiles
    with (
        tc.tile_pool(name="const", bufs=1) as cpool,
        tc.tile_pool(name="wts", bufs=1) as wpool,
        tc.tile_pool(name="annres", bufs=3) as annpool,
        tc.tile_pool(name="annT", bufs=2) as annTpool,
        tc.tile_pool(name="tanh", bufs=2) as tanhpool,
        tc.tile_pool(name="wstage", bufs=2) as wstpool,
        tc.tile_pool(name="small_sb", bufs=2) as smallsb,
    ):
        def issue_ann_dma(b):
            halves = []
            for h in range(2):
                ah = annpool.tile([128, HQ, A], ANT, tag=f"annres{h}")
                nc.gpsimd.dma_start(
                    out=ah,
                    in_=ann_d[b, T // 2 * h:T // 2 * (h + 1), :]
                    .rearrange("(q p) a -> p q a", p=128))
                halves.append(ah)
            return halves

        # annotation DMAs for the first two rows start before anything else
        ann_q = [issue_ann_dma(0), issue_ann_dma(1)]

        ident = cpool.tile([128, 128], F32)
        make_identity(nc, ident)
        ident_t = cpool.tile([128, 128], ANT)
        nc.vector.tensor_copy(ident_t, ident)
        ones11 = cpool.tile([1, 1], F32)
        nc.vector.memset(ones11, 1.0)
        ones11_t = cpool.tile([1, 1], ANT)
        nc.vector.tensor_copy(ones11_t, ones11)
        ones1b_ld = cpool.tile([1, BS], F32)
        nc.vector.memset(ones1b_ld, 1.0)
        ones1b = cpool.tile([1, BS], BF16)
        nc.vector.tensor_copy(ones1b, ones1b_ld)
        half_col = cpool.tile([BS, 1], F32)
        nc.vector.memset(half_col, 0.5)

        # --- replicated weights (fp32 loaders in a transient pool) ---
        ldpool_cm = tc.tile_pool(name="ld", bufs=1)
        ldpool = ldpool_cm.__enter__()
        ku_ld = ldpool.tile([128, J, U], F32)  # ku[a, u] -> [p, j, u], a=128j+p
        nc.sync.dma_start(out=ku_ld, in_=ku_d.rearrange("(j p) u -> p j u", p=128))
        ku_sc = ldpool.tile([128, J, U], F32, name="ku_sc")
        nc.vector.tensor_scalar_mul(ku_sc, ku_ld, KU_SCALE)
        ku_sb = wpool.tile([128, J, U], FP8)
        nc.vector.tensor_copy(ku_sb, ku_sc)
        kw_ld = ldpool.tile([128, J, U], F32)
        nc.sync.dma_start(out=kw_ld, in_=kw_d.rearrange("(j p) u -> p j u", p=128))
        kw_sb = wpool.tile([128, J, U], BF16)
        nc.vector.tensor_copy(kw_sb, kw_ld)
        v_ld = cpool.tile([128, M], F32)       # v[u] -> [p, m], u=128m+p
        nc.sync.dma_start(out=v_ld, in_=kv_d.rearrange("o (m p) -> p (o m)", p=128))
        v_sc = cpool.tile([128, M], F32)
        nc.vector.tensor_scalar_mul(v_sc, v_ld, V_SCALE)
        v_pad = cpool.tile([128, M, 16], FP8)  # fp8 v, 16B-padded k-tile step
        nc.vector.memset(v_pad, 0.0)
        nc.vector.tensor_copy(v_pad[:, :, 0], v_sc)
        biasu_col = cpool.tile([128, M], F32)  # bias[4U:5U] as a column
        nc.sync.dma_start(
            out=biasu_col,
            in_=bias_d[:, 4 * U:5 * U].rearrange("o (m p) -> p (o m)", p=128))
        biasz_ld = cpool.tile([1, 4 * U], F32)
        nc.sync.dma_start(out=biasz_ld, in_=bias_d[:, 0:4 * U])
        biasz_row = cpool.tile([1, 4 * U], BF16)
        nc.vector.tensor_copy(biasz_row, biasz_ld)

        # --- per-core state rows ---
        h_nat = cpool.tile([BS, U], F32)
        nc.sync.dma_start(out=h_nat, in_=h_d)
        in_nat = cpool.tile([BS, D], F32)
        nc.sync.dma_start(out=in_nat, in_=inp_d)
        c_nat = cpool.tile([BS, U], F32)
        nc.sync.dma_start(out=c_nat, in_=c_d)

        hT = wpool.tile([128, M, BS], BF16)      # h^T, contraction layout
        xT = wpool.tile([128, 2 * J, BS], BF16)  # [inputs; context]^T
        bias_att = wpool.tile([128, M, BS], F32)  # Wx^T + bias_u per batch row

        with tc.tile_pool(name="ps_setup", bufs=2, space="PSUM") as pps:
            for j in range(M):
                pt = pps.tile([128, BS], F32)
                nc.tensor.transpose(pt, h_nat[:, 128 * j:128 * (j + 1)],
                                    ident[0:BS, 0:BS])
                nc.vector.tensor_copy(hT[:, j, :], pt)
            for j in range(J):
                pt = pps.tile([128, BS], F32)
                nc.tensor.transpose(pt, in_nat[:, 128 * j:128 * (j + 1)],
                                    ident[0:BS, 0:BS])
                nc.vector.tensor_copy(xT[:, j, :], pt)
            for m in range(M):
                pwx = pps.tile([128, BS], F32)
                for j in range(M):
                    nc.tensor.matmul(pwx,
                                     lhsT=kw_sb[:, j, 128 * m:128 * (m + 1)],
                                     rhs=hT[:, j, :],
                                     start=(j == 0), stop=(j == M - 1))
                nc.scalar.activation(bias_att[:, m, :], pwx, AF.Identity,
                                     bias=biasu_col[:, m:m + 1])
        ldpool_cm.__exit__(None, None, None)

        # LSTM tail weights: fp32 loads on the HWDGE queue (parallel to the
        # Pool/SWDGE queue carrying annotations), converted to bf16 on DVE.
        Wt = wpool.tile([128, 2 * J, 4 * U], BF16)
        Rt = wpool.tile([128, M, 4 * U], BF16)
        w_chunks = []
        for n in range(4):
            for kh in range(2):
                w_chunks.append((Wt[:, 4 * kh:4 * (kh + 1), U * n:U * (n + 1)],
                                 W_d[512 * kh:512 * (kh + 1),
                                     U * n:U * (n + 1)].rearrange(
                                     "(k p) n -> p k n", p=128)))
            w_chunks.append((Rt[:, :, U * n:U * (n + 1)],
                             R_d[:, U * n:U * (n + 1)].rearrange(
                                 "(k p) n -> p k n", p=128)))

        def issue_w_chunk(c):
            dst, src = w_chunks[c]
            st = wstpool.tile([128, M, U], F32, tag="wst")
            nc.sync.dma_start(out=st, in_=src)
            nc.vector.tensor_copy(dst, st)

        # ------------- attention -------------
        with (
            tc.tile_pool(name="stg_ps", bufs=2, space="PSUM") as stgps,
            tc.tile_pool(name="uh_ps", bufs=2, space="PSUM") as uhps,
            tc.tile_pool(name="small_ps", bufs=2, space="PSUM") as smallps,
        ):
            pend = None

            def late_stage(p):
                # et / exp / w-cols / context for batch row b (one row late)
                b, tanhG, ann_halves = p
                denb = smallsb.tile([1, NT], F32, tag="den")
                w_cols = smallsb.tile([128, TS, 16], ANT, tag="wcols")
                for i in range(NT):
                    et_ps = smallps.tile([1, TT], F32, tag="sm")
                    for g in range(M // 2):
                        nc.tensor.matmul(
                            et_ps, lhsT=v_pad[:, 2 * g:2 * g + 2, 0:1],
                            rhs=tanhG[:, 2 * g:2 * g + 2,
                                      TT * i:TT * (i + 1)],
                            start=(g == 0), stop=(g == M // 2 - 1),
                            perf_mode=DR)
                    w_row = smallsb.tile([1, TT], ANT, tag="wrow")
                    nc.scalar.activation(w_row, et_ps, AF.Exp,
                                         scale=1.0 / V_SCALE,
                                         accum_out=denb[:, i:i + 1])
                    wc_ps = smallps.tile([128, NS * 4], ANT, tag="sm")
                    wcw = 4 if ANN_FP8 else 2  # pad cols to 4 bytes
                    for s in range(NS):
                        nc.tensor.transpose(wc_ps[:, wcw * s:wcw * s + 1],
                                            w_row[:, 128 * s:128 * (s + 1)],
                                            ones11_t)
                    nc.vector.tensor_copy(
                        w_cols[:, NS * i:NS * (i + 1), 0],
                        wc_ps.rearrange("p (s w) -> p s w", w=wcw)[:, 0:NS, 0])
                dsum = smallsb.tile([1, 1], F32, tag="dsum")
                nc.vector.reduce_sum(dsum, denb, axis=mybir.AxisListType.X)
                drec = smallsb.tile([1, 1], F32, tag="drec")
                nc.vector.reciprocal(drec, dsum)
                ctx_ps = smallps.tile([1, A], F32, tag="sm")
                if ANN_FP8:
                    for h in range(2):
                        for g in range(HQ // 2):
                            nc.tensor.matmul(
                                ctx_ps,
                                lhsT=w_cols[:, HQ * h + 2 * g:
                                            HQ * h + 2 * g + 2, 0:1],
                                rhs=ann_halves[h][:, 2 * g:2 * g + 2, :],
                                start=(h == 0 and g == 0),
                                stop=(h == 1 and g == HQ // 2 - 1),
                                perf_mode=DR)
                else:
                    for q in range(TS):
                        nc.tensor.matmul(
                            ctx_ps, lhsT=w_cols[:, q, 0:1],
                            rhs=ann_halves[q // HQ][:, q % HQ, :],
                            start=(q == 0), stop=(q == TS - 1))
                ctx_row = smallsb.tile([1, A], F32, tag="ctxrow")
                nc.vector.tensor_scalar_mul(ctx_row, ctx_ps, drec)
                cT_ps = smallps.tile([128, J], F32, tag="sm")
                for j in range(J):
                    nc.tensor.transpose(cT_ps[:, j:j + 1],
                                        ctx_row[:, 128 * j:128 * (j + 1)],
                                        ones11)
                nc.vector.tensor_copy(xT[:, J:2 * J, b], cT_ps)

            for b in range(BS):
                ann_halves = ann_q.pop(0)
                if b + 2 < BS:
                    ann_q.append(issue_ann_dma(b + 2))
                for c in (2 * b, 2 * b + 1):
                    if c < len(w_chunks):
                        issue_w_chunk(c)

                # Per half-row: transpose burst then uh/tanh burst, so plain
                # matmuls pulse on PE at a sub-3.4us cadence (HAM warmth).
                # Transposes are REGULAR bf16 matmuls (out = ann_chunk.T @ I)
                # rather than transpose-mode, which the HAM activity monitor
                # does not count as PE-busy.
                annT = annTpool.tile([128, J, T], FP8)
                tanhG = tanhpool.tile([128, M, T], FP8)
                for h in range(2):
                    for i in range(HQ // NS):
                        for j in range(J):
                            stg = stgps.tile([128, TT], F32, tag="stg")
                            for s in range(NS):
                                nc.tensor.matmul(
                                    stg[:, 128 * s:128 * (s + 1)],
                                    lhsT=ann_halves[h][:, NS * i + s,
                                                       128 * j:128 * (j + 1)],
                                    rhs=ident_t,
                                    start=True, stop=True)
                            nc.vector.tensor_copy(
                                annT[:, j, T // 2 * h + TT * i:
                                     T // 2 * h + TT * (i + 1)], stg)
                    for m in range(M):
                        uh = uhps.tile([128, HB], F32, tag="uh")
                        for c in range(2):
                            for g in range(J // 2):
                                nc.tensor.matmul(
                                    uh[:, TT * c:TT * (c + 1)],
                                    lhsT=ku_sb[:, 2 * g:2 * g + 2,
                                               128 * m:128 * (m + 1)],
                                    rhs=annT[:, 2 * g:2 * g + 2,
                                             HB * h + TT * c:
                                             HB * h + TT * (c + 1)],
                                    start=(g == 0), stop=(g == J // 2 - 1),
                                    perf_mode=DR)
                        nc.scalar.activation(
                            tanhG[:, m, HB * h:HB * (h + 1)], uh, AF.Tanh,
                            bias=bias_att[:, m, b:b + 1],
                            scale=1.0 / KU_SCALE)
                    if h == 0 and pend is not None:
                        late_stage(pend)
                        pend = None

                pend = (b, tanhG, ann_halves)

            late_stage(pend)

        # ------------- LSTM tail -------------
        with (
            tc.tile_pool(name="z_ps", bufs=2, space="PSUM") as zpool,
            tc.tile_pool(name="gates", bufs=1) as gpool,
        ):
            gates = []
            for n in range(4):
                zps = zpool.tile([BS, U], F32)
                for k in range(2 * J):
                    nc.tensor.matmul(zps, lhsT=xT[:, k, :],
                                     rhs=Wt[:, k, U * n:U * (n + 1)],
                                     start=(k == 0), stop=False)
                for k in range(M):
                    nc.tensor.matmul(zps, lhsT=hT[:, k, :],
                                     rhs=Rt[:, k, U * n:U * (n + 1)],
                                     start=False, stop=False)
                nc.tensor.matmul(zps, lhsT=ones1b,
                                 rhs=biasz_row[:, U * n:U * (n + 1)],
                                 start=False, stop=True)
                g = gpool.tile([BS, U], F32, tag=f"gate{n}")
                if n == 2:
                    nc.scalar.activation(g, zps, AF.Tanh)
                else:
                    nc.scalar.activation(g, zps, AF.Relu, bias=half_col,
                                         scale=0.2)
                    nc.vector.tensor_scalar_min(g, g, 1.0)
                gates.append(g)

            gi, gf, gg, go = gates
            c_new = gpool.tile([BS, U], F32, tag="cnew")
            nc.vector.tensor_mul(c_new, gf, c_nat)
            ig = gpool.tile([BS, U], F32, tag="ig")
            nc.vector.tensor_mul(ig, gi, gg)
            nc.vector.tensor_add(c_new, c_new, ig)
            tc_t = gpool.tile([BS, U], F32, tag="tanhc")
            nc.scalar.activation(tc_t, c_new, AF.Tanh)
            h_new = gpool.tile([BS, U], F32, tag="hnew")
            nc.vector.tensor_mul(h_new, go, tc_t)
            nc.sync.dma_start(out=out_d, in_=h_new)


_NC_CACHE = None


def _get_nc():
    global _NC_CACHE
    if _NC_CACHE is None:
        _NC_CACHE = build_bass()
    return _NC_CACHE


def make_in_maps(inputs, h, c, annotations, kernel, recurrent_kernel, bias,
                 kernel_u, kernel_w, kernel_v):
    asc = np.ascontiguousarray
    maps = []
    for core in range(N_CORES):
        sl = slice(core * BS, (core + 1) * BS)
        maps.append({
            "ann": asc(annotations[sl]).astype(np.float32),
            "inputs": asc(inputs[sl]).astype(np.float32),
            "h": asc(h[sl]).astype(np.float32),
            "c": asc(c[sl]).astype(np.float32),
            "kernel": asc(kernel).astype(np.float32),
            "rkernel": asc(recurrent_kernel).astype(np.float32),
            "bias": asc(bias).reshape(1, 6 * U).astype(np.float32),
            "ku": asc(kernel_u).astype(np.float32),
            "kw": asc(kernel_w).astype(np.float32),
            "kv": asc(kernel_v).reshape(1, U).astype(np.float32),
        })
    return maps


def kernel(inputs, h, c, annotations, kernel, recurrent_kernel, bias,
           kernel_u, kernel_w, kernel_v, _trace=False):
    nc = _get_nc()
    in_maps = make_in_maps(inputs, h, c, annotations, kernel,
                           recurrent_kernel, bias, kernel_u, kernel_w,
                           kernel_v)
    res = run_bass_kernel_spmd(nc, in_maps, list(range(N_CORES)),
                               trace=_trace)
    out = np.concatenate([res.results[i]["out"] for i in range(N_CORES)],
                         axis=0)
    if _trace:
        globals()["last_exec_time_ns"] = res.exec_time_ns
        globals()["last_results"] = res
    return out
